# revision 1
# baseline (speedup 1.0000x reference)
"""HGNN layer (hypergraph message passing) Trainium2 kernel, 8 NeuronCores.

Sharding: one graph per PAIR of cores (4 graphs x 2 cores). Within a pair
each core owns half the hyperedge/node range. Matmuls keep the big matrix
as the MOVING operand (free dim 512, fp32r / bf16) and the [4096,128]
intermediate as the stationary operand, so every big matrix streams from
HBM once at line rate in the layout the PE needs (the host supplies
transposed shards where the PE requires contraction-major layout).
Intermediates flow in "transposed" [128, 4096] form; PE transposes
convert back to contraction-major tiles between stages. 3 pair-AllReduces
merge the split contractions. Softmax is computed unnormalized; 1/Z is
folded in after the first AllReduce (Z rides along in the collective
buffer).
"""

import numpy as np

B, N, E, D = 4, 4096, 4096, 128
HALF = N // 2
NCORES = 8
PAIRS = [[0, 1], [2, 3], [4, 5], [6, 7]]
BN_EPS = 1e-5
F = 512                 # moving free-dim per matmul
NT = N // 128           # 32 k-tiles over a full 4096 dim
HT = HALF // 128        # 16 k-tiles over a half
RESIDENT_N = 6          # how many of the 16 Ht bf16 tiles stay SBUF-resident

_CACHE = {}


def _build():
    import concourse.bacc as bacc
    import concourse.mybir as mybir
    import concourse.tile as tile
    from concourse.masks import make_identity
    from contextlib import ExitStack

    fp32 = mybir.dt.float32
    fp32r = mybir.dt.float32r
    bf16 = mybir.dt.bfloat16
    Act = mybir.ActivationFunctionType
    Alu = mybir.AluOpType

    nc = bacc.Bacc("TRN2", target_bir_lowering=False, debug=False,
                   num_devices=NCORES)

    # ---- per-core DRAM inputs (shards; see kernel() for host layout) ----
    xT_d = nc.dram_tensor("xT", [D, N], fp32, kind="ExternalInput")
    hcol_d = nc.dram_tensor("hcol", [N, HALF], fp32, kind="ExternalInput")
    htr_d = nc.dram_tensor("htr", [HALF, N], fp32, kind="ExternalInput")
    hrow_d = nc.dram_tensor("hrow", [HALF, N], fp32, kind="ExternalInput")
    dvT_d = nc.dram_tensor("dvT", [N, HALF], fp32, kind="ExternalInput")
    deT_d = nc.dram_tensor("deT", [N, HALF], fp32, kind="ExternalInput")
    w_d = nc.dram_tensor("w", [D, D], fp32, kind="ExternalInput")
    b_d = nc.dram_tensor("b", [D, 1], fp32, kind="ExternalInput")
    th_d = nc.dram_tensor("th", [D, 1], fp32, kind="ExternalInput")
    mask_d = nc.dram_tensor("mask", [1, HALF], fp32, kind="ExternalInput")
    eps_d = nc.dram_tensor("eps", [D, 1], fp32, kind="ExternalInput")
    bng_d = nc.dram_tensor("bng", [D, 1], fp32, kind="ExternalInput")
    bnb_d = nc.dram_tensor("bnb", [D, 1], fp32, kind="ExternalInput")
    bnm_d = nc.dram_tensor("bnm", [D, 1], fp32, kind="ExternalInput")
    bnv_d = nc.dram_tensor("bnv", [D, 1], fp32, kind="ExternalInput")
    y_d = nc.dram_tensor("y", [D, N], fp32, kind="ExternalOutput")

    def r(ap):
        return ap.bitcast(fp32r)

    with tile.TileContext(nc) as tc, ExitStack() as ctx:
        const = ctx.enter_context(tc.tile_pool(name="const", bufs=1))
        resident = ctx.enter_context(tc.tile_pool(name="resident", bufs=1))
        stream = ctx.enter_context(tc.tile_pool(name="stream", bufs=4))
        streamb = ctx.enter_context(tc.tile_pool(name="streamb", bufs=2))
        big = ctx.enter_context(tc.tile_pool(name="big", bufs=1))
        med = ctx.enter_context(tc.tile_pool(name="med", bufs=1))
        small = ctx.enter_context(tc.tile_pool(name="small", bufs=1))
        ps = ctx.enter_context(tc.tile_pool(name="ps", bufs=8, space="PSUM"))
        dram = ctx.enter_context(tc.tile_pool(name="dram", bufs=1, space="DRAM"))

        ident = const.tile([128, 128], fp32)
        make_identity(nc, ident)
        one11 = const.tile([1, 1], fp32)
        nc.vector.memset(one11[:], 1.0)
        ones_row = const.tile([1, 128], fp32)
        nc.vector.memset(ones_row[:], 1.0)
        ones2 = const.tile([2, 1], fp32)
        nc.vector.memset(ones2[:], 1.0)

        def load_param(dt_):
            t = const.tile([D, 1], fp32, tag=dt_.name + "_p")
            nc.sync.dma_start(out=t[:], in_=dt_.ap())
            return t

        w_t = const.tile([D, D], fp32)
        nc.sync.dma_start(out=w_t[:], in_=w_d.ap())
        b_t = load_param(b_d)
        th_t = load_param(th_d)
        eps_t = load_param(eps_d)
        bng_t = load_param(bng_d)
        bnb_t = load_param(bnb_d)
        bnm_t = load_param(bnm_d)
        bnv_t = load_param(bnv_d)
        mask_t = const.tile([1, HALF], fp32)
        nc.sync.dma_start(out=mask_t[:], in_=mask_d.ap())

        # resident Ht bf16 tiles [128e, N] (first RESIDENT_N of HT tiles),
        # loaded once via SWDGE cast-DMA; used by stages 6 and 11.
        ht_res = resident.tile([128, RESIDENT_N * N], bf16)
        for t in range(RESIDENT_N):
            nc.gpsimd.dma_start(
                out=ht_res[:, t * N:(t + 1) * N],
                in_=htr_d.ap()[t * 128:(t + 1) * 128, :])

        def ht_tile(t, tag):
            if t < RESIDENT_N:
                return ht_res[:, t * N:(t + 1) * N]
            tt = streamb.tile([128, N], bf16, tag="htstream", name="htt")
            nc.gpsimd.dma_start(
                out=tt[:], in_=htr_d.ap()[t * 128:(t + 1) * 128, :])
            return tt[:]

        def transpose_cols(src, j, out_ap, scale=None, w128=128):
            """PE-transpose src[:, 128j:128j+128] -> out_ap (optionally
            scaled per-partition by `scale` [128,1]) via psum."""
            pt = ps.tile([128, 128], fp32, tag="ps")
            nc.tensor.transpose(pt[:, 0:w128], src[:, j * 128:j * 128 + w128],
                                ident[:])
            if scale is None:
                nc.vector.tensor_copy(out_ap, pt[:, 0:w128])
            else:
                nc.vector.tensor_scalar_mul(out_ap, pt[:, 0:w128], scale)

        # ------- stage 1: x_wT = (x@W+b).T [D,N]; xthT = (x@th).T [1,N] ----
        xT_t = big.tile([D, N], fp32, tag="bigA")
        nc.sync.dma_start(out=xT_t[:], in_=xT_d.ap())
        x_wT = big.tile([D, N], fp32, tag="bigB")
        xthT = small.tile([1, N], fp32, tag="xthT")
        for blk in range(N // F):
            sl = slice(blk * F, (blk + 1) * F)
            p1 = ps.tile([128, F], fp32, tag="ps")
            nc.tensor.matmul(p1[:], w_t[:], xT_t[:, sl],
                             start=True, stop=True)
            nc.vector.tensor_scalar_add(x_wT[:, sl], p1[:], b_t[:])
            p2 = ps.tile([1, F], fp32, tag="ps")
            nc.tensor.matmul(p2[:], th_t[:], xT_t[:, sl],
                             start=True, stop=True)
            nc.vector.tensor_copy(xthT[:, sl], p2[:])

        # x_w vN tiles [128n, 128d] packed as x_wv[:, j*128:...] and
        # xth vN columns [128n, 1] packed as xthv[:, j]
        x_wv = med.tile([D, N], bf16, tag="x_wv")
        for j in range(NT):
            transpose_cols(x_wT[:], j, x_wv[:, j * 128:(j + 1) * 128])
        xthv = med.tile([128, 2 * NT], bf16, tag="xthv")
        xthv32 = med.tile([128, 1], fp32, tag="xthv32")
        for j in range(NT):
            pt = ps.tile([128, 1], fp32, tag="ps")
            nc.tensor.matmul(pt[:], xthT[:, j * 128:(j + 1) * 128], one11[:],
                             start=True, stop=True)
            # hi/lo bf16 split so the attention scores keep ~fp32 accuracy
            nc.vector.tensor_copy(xthv[:, 2 * j:2 * j + 1], pt[:])
            nc.vector.tensor_tensor(xthv32[:], pt[:], xthv[:, 2 * j:2 * j + 1],
                                    op=Alu.subtract)
            nc.vector.tensor_copy(xthv[:, 2 * j + 1:2 * j + 2], xthv32[:])

        # ------- stage 2: hxT[d, e_half] = (Ht@x_w).T ; sth[1, e_half] ----
        hx_ps = [ps.tile([128, F], fp32, tag="ps", name=f"hx_ps{i}") for i in range(HALF // F)]
        st_ps = [ps.tile([2, F], fp32, tag="ps", name=f"st_ps{i}") for i in range(HALF // F)]
        for j in range(NT):
            hj = stream.tile([128, HALF], bf16, tag="stream")
            nc.gpsimd.dma_start(out=hj[:],
                                in_=hcol_d.ap()[j * 128:(j + 1) * 128, :])
            for blk in range(HALF // F):
                sl = slice(blk * F, (blk + 1) * F)
                nc.tensor.matmul(hx_ps[blk][:],
                                 x_wv[:, j * 128:(j + 1) * 128],
                                 hj[:, sl],
                                 start=(j == 0), stop=(j == NT - 1))
                nc.tensor.matmul(st_ps[blk][:], xthv[:, 2 * j:2 * j + 2],
                                 hj[:, sl],
                                 start=(j == 0), stop=(j == NT - 1))
        hxT = med.tile([D, HALF], fp32, tag="hxT")
        sth = small.tile([1, HALF], fp32, tag="sth")
        for blk in range(HALF // F):
            sl = slice(blk * F, (blk + 1) * F)
            nc.vector.tensor_copy(hxT[:, sl], hx_ps[blk][:])
            s2sb = med.tile([2, F], fp32, tag="s2sb", name=f"s2sb{blk}")
            nc.vector.tensor_copy(s2sb[:], st_ps[blk][0:2, :])
            sp = ps.tile([1, F], fp32, tag="ps", name=f"sp{blk}")
            nc.tensor.matmul(sp[:], ones2[:], s2sb[:], start=True, stop=True)
            nc.vector.tensor_copy(sth[:, sl], sp[:])

        # ------- softmax pieces: attn_u = exp(sth)*mask ; z = sum(attn_u) --
        attn_u = small.tile([1, HALF], fp32, tag="attn_u")
        nc.scalar.activation(attn_u[:], sth[:], Act.Exp)
        nc.vector.tensor_mul(attn_u[:], attn_u[:], mask_t[:])
        z_t = small.tile([1, 1], fp32, tag="z_t")
        nc.vector.reduce_sum(z_t[:], attn_u[:], axis=mybir.AxisListType.X)
        # attn as per-partition columns attnv[:, t]
        attnv = med.tile([128, HT], fp32, tag="attnv")
        for t in range(HT):
            pt = ps.tile([128, 1], fp32, tag="ps")
            nc.tensor.matmul(pt[:], attn_u[:, t * 128:(t + 1) * 128], one11[:],
                             start=True, stop=True)
            nc.vector.tensor_copy(attnv[:, t:t + 1], pt[:])
        # eps-scaled hxT for stage 10
        ehxT = med.tile([D, HALF], fp32, tag="ehxT")
        nc.vector.tensor_scalar_mul(ehxT[:], hxT[:], eps_t[:])

        # ------- h1a vE tiles (bf16): h1a[:, t] = attn*hx tile t ----------
        h1a = med.tile([128, HALF], bf16, tag="h1a")
        for t in range(HT):
            pt = ps.tile([128, 128], fp32, tag="ps")
            nc.tensor.transpose(pt[:], hxT[:, t * 128:(t + 1) * 128], ident[:])
            nc.vector.tensor_scalar_mul(h1a[:, t * 128:(t + 1) * 128], pt[:],
                                        attnv[:, t:t + 1])

        # ------- stage 6: h1bT_part [D, N] = (H @ h1a)_partial.T ----------
        h1b_ps = [ps.tile([128, F], fp32, tag="ps", name=f"h1b_ps{i}") for i in range(N // F)]
        for t in range(HT):
            htt = ht_tile(t, "s6")
            for blk in range(N // F):
                sl = slice(blk * F, (blk + 1) * F)
                nc.tensor.matmul(h1b_ps[blk][:],
                                 h1a[:, t * 128:(t + 1) * 128], htt[:, sl],
                                 start=(t == 0), stop=(t == HT - 1))
        # evict with z riding in col N (cols N..N+7 zeroed)
        cc1_sb = big.tile([D, N + 8], fp32, tag="bigA")
        for blk in range(N // F):
            sl = slice(blk * F, (blk + 1) * F)
            nc.vector.tensor_copy(cc1_sb[:, sl], h1b_ps[blk][:])
        nc.vector.memset(cc1_sb[:, N:], 0.0)
        nc.vector.tensor_copy(cc1_sb[0:1, N:N + 1], z_t[:])
        cc1_in = dram.tile([D, N + 8], fp32, tag="cc1i")
        cc1_out = dram.tile([D, N + 8], fp32, tag="cc1o")
        nc.sync.dma_start(out=cc1_in[:], in_=cc1_sb[:])
        nc.gpsimd.collective_compute(
            "AllReduce", Alu.add, replica_groups=PAIRS,
            ins=[cc1_in.opt()], outs=[cc1_out.opt()])
        h1b_full = big.tile([D, N + 8], fp32, tag="bigB")
        nc.sync.dma_start(out=h1b_full[:], in_=cc1_out[:])

        # 1/z broadcast to [128, 1]
        rz = small.tile([1, 1], fp32, tag="rz")
        nc.vector.reciprocal(rz[:], h1b_full[0:1, N:N + 1])
        rz_ps = ps.tile([128, 1], fp32, tag="ps")
        nc.tensor.matmul(rz_ps[:], ones_row[:], rz[:], start=True, stop=True)
        rz_bc = small.tile([128, 1], fp32, tag="rz_bc")
        nc.vector.tensor_copy(rz_bc[:], rz_ps[:])

        # h1b vN tiles scaled by 1/z
        h1bv = med.tile([D, N], bf16, tag="x_wv")
        for j in range(NT):
            transpose_cols(h1b_full[:], j, h1bv[:, j * 128:(j + 1) * 128],
                           scale=rz_bc[:])

        # ------- stage 7: h1cT [D, HALF] = (Dv @ h1b).T rows-half ---------
        h1c_ps = [ps.tile([128, F], fp32, tag="ps", name=f"h1c_ps{i}") for i in range(HALF // F)]
        for j in range(NT):
            dj = stream.tile([128, HALF], bf16, tag="stream")
            nc.gpsimd.dma_start(out=dj[:],
                                in_=dvT_d.ap()[j * 128:(j + 1) * 128, :])
            for blk in range(HALF // F):
                sl = slice(blk * F, (blk + 1) * F)
                nc.tensor.matmul(h1c_ps[blk][:],
                                 h1bv[:, j * 128:(j + 1) * 128],
                                 dj[:, sl],
                                 start=(j == 0), stop=(j == NT - 1))
        h1cT = med.tile([D, HALF], fp32, tag="hxT2")
        for blk in range(HALF // F):
            sl = slice(blk * F, (blk + 1) * F)
            nc.vector.tensor_copy(h1cT[:, sl], h1c_ps[blk][:])

        # h1c vN tiles
        h1cv = med.tile([D, HALF], bf16, tag="h1cv")
        for t in range(HT):
            transpose_cols(h1cT[:], t, h1cv[:, t * 128:(t + 1) * 128])

        # ------- stage 8: h1dT_part [D, N] = (Ht @ h1c)_partial.T ---------
        h1d_ps = [ps.tile([128, F], fp32, tag="ps", name=f"h1d_ps{i}") for i in range(N // F)]
        for t in range(HT):
            rj1 = stream.tile([128, HALF], bf16, tag="stream", name="rj1")
            nc.gpsimd.dma_start(out=rj1[:],
                                in_=hrow_d.ap()[t * 128:(t + 1) * 128, 0:HALF])
            rj2 = stream.tile([128, HALF], bf16, tag="stream", name="rj2")
            nc.gpsimd.dma_start(out=rj2[:],
                                in_=hrow_d.ap()[t * 128:(t + 1) * 128, HALF:N])
            for blk in range(N // F):
                sl = slice((blk % 4) * F, (blk % 4 + 1) * F)
                rj = rj1 if blk < 4 else rj2
                nc.tensor.matmul(h1d_ps[blk][:],
                                 h1cv[:, t * 128:(t + 1) * 128],
                                 rj[:, sl],
                                 start=(t == 0), stop=(t == HT - 1))
        cc2_sb = big.tile([D, N], fp32, tag="bigA")
        for blk in range(N // F):
            sl = slice(blk * F, (blk + 1) * F)
            nc.vector.tensor_copy(cc2_sb[:, sl], h1d_ps[blk][:])
        cc2_in = dram.tile([D, N], fp32, tag="cc2i")
        cc2_out = dram.tile([D, N], fp32, tag="cc2o")
        nc.sync.dma_start(out=cc2_in[:], in_=cc2_sb[:])
        nc.gpsimd.collective_compute(
            "AllReduce", Alu.add, replica_groups=PAIRS,
            ins=[cc2_in.opt()], outs=[cc2_out.opt()])
        h1d_full = big.tile([D, N], fp32, tag="bigB")
        nc.sync.dma_start(out=h1d_full[:], in_=cc2_out[:])

        # h1d vE tiles
        h1dv = med.tile([D, N], bf16, tag="x_wv")
        for j in range(NT):
            transpose_cols(h1d_full[:], j, h1dv[:, j * 128:(j + 1) * 128])

        # ------- stage 9: h1eT [D, HALF] = (De @ h1d).T rows-half ---------
        h1e_ps = [ps.tile([128, F], fp32, tag="ps", name=f"h1e_ps{i}") for i in range(HALF // F)]
        for j in range(NT):
            ej = stream.tile([128, HALF], bf16, tag="stream")
            nc.gpsimd.dma_start(out=ej[:],
                                in_=deT_d.ap()[j * 128:(j + 1) * 128, :])
            for blk in range(HALF // F):
                sl = slice(blk * F, (blk + 1) * F)
                nc.tensor.matmul(h1e_ps[blk][:],
                                 h1dv[:, j * 128:(j + 1) * 128],
                                 ej[:, sl],
                                 start=(j == 0), stop=(j == NT - 1))
        # ------- stage 10: hT = h1eT + eps*hxT ; hv bf16 tiles ------------
        hT = med.tile([D, HALF], fp32, tag="hxT2b")
        for blk in range(HALF // F):
            sl = slice(blk * F, (blk + 1) * F)
            nc.vector.tensor_tensor(hT[:, sl], h1e_ps[blk][:], ehxT[:, sl],
                                    op=Alu.add)
        hv = med.tile([128, HALF], bf16, tag="h1a")
        for t in range(HT):
            pt = ps.tile([128, 128], fp32, tag="ps")
            nc.tensor.transpose(pt[:], hT[:, t * 128:(t + 1) * 128], ident[:])
            nc.vector.tensor_copy(hv[:, t * 128:(t + 1) * 128], pt[:])

        # ------- stage 11: outT_part [D, N] = (H @ h)_partial.T -----------
        out_ps = [ps.tile([128, F], fp32, tag="ps", name=f"out_ps{i}") for i in range(N // F)]
        for t in range(HT):
            htt = ht_tile(t, "s11")
            for blk in range(N // F):
                sl = slice(blk * F, (blk + 1) * F)
                nc.tensor.matmul(out_ps[blk][:],
                                 hv[:, t * 128:(t + 1) * 128], htt[:, sl],
                                 start=(t == 0), stop=(t == HT - 1))
        cc3_sb = big.tile([D, N], fp32, tag="bigA")
        for blk in range(N // F):
            sl = slice(blk * F, (blk + 1) * F)
            nc.vector.tensor_copy(cc3_sb[:, sl], out_ps[blk][:])
        cc3_in = dram.tile([D, N], fp32, tag="cc3i")
        cc3_out = dram.tile([D, N], fp32, tag="cc3o")
        nc.sync.dma_start(out=cc3_in[:], in_=cc3_sb[:])
        nc.gpsimd.collective_compute(
            "AllReduce", Alu.add, replica_groups=PAIRS,
            ins=[cc3_in.opt()], outs=[cc3_out.opt()])
        outT = big.tile([D, N], fp32, tag="bigB")
        nc.sync.dma_start(out=outT[:], in_=cc3_out[:])

        # ------- stage 12: epilogue: bn(leaky_relu(outT)) -----------------
        # bn scale s = gamma * rsqrt(var + eps_bn); shift t = beta - mean*s
        s_bn = small.tile([D, 1], fp32, tag="s_bn")
        nc.vector.tensor_scalar_add(s_bn[:], bnv_t[:], BN_EPS)
        nc.scalar.activation(s_bn[:], s_bn[:], Act.Sqrt)
        nc.vector.reciprocal(s_bn[:], s_bn[:])
        nc.vector.tensor_mul(s_bn[:], s_bn[:], bng_t[:])
        t_bn = small.tile([D, 1], fp32, tag="t_bn")
        nc.vector.tensor_mul(t_bn[:], bnm_t[:], s_bn[:])
        nc.vector.tensor_tensor(t_bn[:], bnb_t[:], t_bn[:],
                                op=Alu.subtract)
        nc.scalar.activation(outT[:], outT[:], Act.Lrelu, alpha=0.01)
        nc.vector.tensor_scalar(outT[:], outT[:], s_bn[:], t_bn[:],
                                op0=Alu.mult, op1=Alu.add)
        nc.sync.dma_start(out=y_d.ap(), in_=outT[:])

    nc.finalize()
    return nc


def _get_nc():
    if "nc" not in _CACHE:
        _CACHE["nc"] = _build()
    return _CACHE["nc"]


def _shard(inputs):
    H = np.asarray(inputs["incident_mat"], dtype=np.float32)
    Dv = np.asarray(inputs["degree_v"], dtype=np.float32)
    De = np.asarray(inputs["degree_e"], dtype=np.float32)
    x = np.asarray(inputs["x"], dtype=np.float32)
    em = np.asarray(inputs["e_masks"])
    w = np.ascontiguousarray(np.asarray(inputs["mlp_W"], dtype=np.float32))
    b = np.ascontiguousarray(
        np.asarray(inputs["mlp_b"], dtype=np.float32).reshape(D, 1))
    th = np.ascontiguousarray(
        np.asarray(inputs["theta_att"], dtype=np.float32).reshape(D, 1))
    eps = np.full((D, 1), float(np.asarray(inputs["eps"]).reshape(-1)[0]),
                  dtype=np.float32)

    def col(v):
        return np.ascontiguousarray(
            np.asarray(v, dtype=np.float32).reshape(D, 1))

    bng, bnb = col(inputs["bn_gamma"]), col(inputs["bn_beta"])
    bnm, bnv = col(inputs["bn_mean"]), col(inputs["bn_var"])

    in_maps = []
    for core in range(NCORES):
        g, c = core // 2, core % 2
        lo, hi = c * HALF, (c + 1) * HALF
        Hg = H[g]
        htr = np.ascontiguousarray(Hg.T[lo:hi, :])
        in_maps.append({
            "xT": np.ascontiguousarray(x[g].T),
            "hcol": np.ascontiguousarray(Hg[:, lo:hi]),
            "htr": htr,
            "hrow": np.ascontiguousarray(Hg[lo:hi, :]),
            "dvT": np.ascontiguousarray(Dv[g][lo:hi, :].T),
            "deT": np.ascontiguousarray(De[g][lo:hi, :].T),
            "w": w, "b": b, "th": th,
            "mask": np.ascontiguousarray(
                em[g, lo:hi].astype(np.float32).reshape(1, HALF)),
            "eps": eps,
            "bng": bng, "bnb": bnb, "bnm": bnm, "bnv": bnv,
        })
    return in_maps


def kernel(**inputs):
    from concourse.bass_utils import run_bass_kernel_spmd

    nc = _get_nc()
    in_maps = _shard(inputs)
    res = run_bass_kernel_spmd(nc, in_maps, list(range(NCORES)))
    out = np.empty((B, N, D), dtype=np.float32)
    for g in range(B):
        ya = res.results[2 * g]["y"]
        yb = res.results[2 * g + 1]["y"]
        out[g, :HALF, :] = ya[:, :HALF].T
        out[g, HALF:, :] = yb[:, HALF:].T
    return out



# revision 2
# speedup vs baseline: 1.7654x; 1.7654x over previous
"""HGNN layer (hypergraph message passing) Trainium2 kernel, 8 NeuronCores.

Sharding: one graph per PAIR of cores (4 graphs x 2 cores), output-split:
within a pair each core owns HALF of every stage's output rows/columns
(e-half for Ht@/De@ stages, n-half for H@/Dv@ stages). Each stage streams
its big-matrix shard (bf16, cast on host) as the MOVING matmul operand in
2MB chunks over HWDGE while the [*,128] intermediate sits stationary in
bf16 vN/vE tile form. Between stages, the pair exchanges the half-outputs
with a 0.5MB AllGather (the softmax normalizer z rides along in the first
exchange as a hi/lo bf16 pair). Only 4 distinct big tensors are uploaded
per core (hcol, htc, dvT, deT); hcol and htc are each streamed twice.
"""

import numpy as np

B, N, E, D = 4, 4096, 4096, 128
HALF = N // 2
NCORES = 8
PAIRS = [[0, 1], [2, 3], [4, 5], [6, 7]]
BN_EPS = 1e-5
F = 512                 # moving free-dim per matmul
NT = N // 128           # 32 k-tiles over a full 4096 dim
HT = HALF // 128        # 16 tiles over a half (own output)
CH = 4                  # k-tiles per DMA chunk (2MB bf16)
NCHUNK = NT // CH       # 8 chunks per streamed matrix
ZW = 8                  # rider columns appended to exchange 1

_CACHE = {}


def _build():
    import concourse.bacc as bacc
    import concourse.mybir as mybir
    import concourse.tile as tile
    from concourse.masks import make_identity
    from contextlib import ExitStack

    fp32 = mybir.dt.float32
    bf16 = mybir.dt.bfloat16
    Act = mybir.ActivationFunctionType
    Alu = mybir.AluOpType

    nc = bacc.Bacc("TRN2", target_bir_lowering=False, debug=False,
                   num_devices=NCORES)

    # ---- per-core DRAM inputs (shards; see kernel() for host layout) ----
    xT_d = nc.dram_tensor("xT", [D, N], bf16, kind="ExternalInput")
    hcol_d = nc.dram_tensor("hcol", [N, HALF], bf16, kind="ExternalInput")
    htc_d = nc.dram_tensor("htc", [E, HALF], bf16, kind="ExternalInput")
    dvT_d = nc.dram_tensor("dvT", [N, HALF], bf16, kind="ExternalInput")
    deT_d = nc.dram_tensor("deT", [E, HALF], bf16, kind="ExternalInput")
    w_d = nc.dram_tensor("w", [D, D], bf16, kind="ExternalInput")
    b_d = nc.dram_tensor("b", [D, 1], fp32, kind="ExternalInput")
    th_d = nc.dram_tensor("th", [D, 1], bf16, kind="ExternalInput")
    mask_d = nc.dram_tensor("mask", [1, HALF], fp32, kind="ExternalInput")
    eps_d = nc.dram_tensor("eps", [D, 1], fp32, kind="ExternalInput")
    bng_d = nc.dram_tensor("bng", [D, 1], fp32, kind="ExternalInput")
    bnb_d = nc.dram_tensor("bnb", [D, 1], fp32, kind="ExternalInput")
    bnm_d = nc.dram_tensor("bnm", [D, 1], fp32, kind="ExternalInput")
    bnv_d = nc.dram_tensor("bnv", [D, 1], fp32, kind="ExternalInput")
    y_d = nc.dram_tensor("y", [D, HALF], fp32, kind="ExternalOutput")

    with tile.TileContext(nc) as tc, ExitStack() as ctx:
        const = ctx.enter_context(tc.tile_pool(name="const", bufs=1))
        stream = ctx.enter_context(tc.tile_pool(name="stream", bufs=3))
        med = ctx.enter_context(tc.tile_pool(name="med", bufs=1))
        small = ctx.enter_context(tc.tile_pool(name="small", bufs=1))
        ps = ctx.enter_context(tc.tile_pool(name="ps", bufs=8, space="PSUM"))
        dram = ctx.enter_context(tc.tile_pool(name="dram", bufs=1, space="DRAM"))

        ident = const.tile([128, 128], fp32)
        make_identity(nc, ident)
        one11 = const.tile([1, 1], fp32)
        nc.vector.memset(one11[:], 1.0)
        ones_row = const.tile([1, 128], fp32)
        nc.vector.memset(ones_row[:], 1.0)
        ones2 = const.tile([2, 1], fp32)
        nc.vector.memset(ones2[:], 1.0)

        def load_param(dt_, dt=fp32):
            t = const.tile([D, 1], dt, tag=dt_.name + "_p")
            nc.sync.dma_start(out=t[:], in_=dt_.ap())
            return t

        w_t = const.tile([D, D], bf16)
        nc.sync.dma_start(out=w_t[:], in_=w_d.ap())
        b_t = load_param(b_d)
        th_t = load_param(th_d, bf16)
        eps_t = load_param(eps_d)
        bng_t = load_param(bng_d)
        bnb_t = load_param(bnb_d)
        bnm_t = load_param(bnm_d)
        bnv_t = load_param(bnv_d)
        mask_t = const.tile([1, HALF], fp32)
        nc.sync.dma_start(out=mask_t[:], in_=mask_d.ap())

        # bn scale s = gamma * rsqrt(var + eps_bn); shift t = beta - mean*s
        s_bn = small.tile([D, 1], fp32, tag="s_bn")
        nc.vector.tensor_scalar_add(s_bn[:], bnv_t[:], BN_EPS)
        nc.scalar.activation(s_bn[:], s_bn[:], Act.Sqrt)
        nc.vector.reciprocal(s_bn[:], s_bn[:])
        nc.vector.tensor_mul(s_bn[:], s_bn[:], bng_t[:])
        t_bn = small.tile([D, 1], fp32, tag="t_bn")
        nc.vector.tensor_mul(t_bn[:], bnm_t[:], s_bn[:])
        nc.vector.tensor_tensor(t_bn[:], bnb_t[:], t_bn[:], op=Alu.subtract)

        def transpose_cols(src, j, out_ap, scale=None):
            """PE-transpose src[:, 128j:128(j+1)] -> out_ap (optionally
            scaled per-partition by `scale` [128,1]) via psum."""
            pt = ps.tile([128, 128], fp32, tag="ps", name="pt")
            nc.tensor.transpose(pt[:], src[:, j * 128:(j + 1) * 128], ident[:])
            if scale is None:
                nc.vector.tensor_copy(out_ap, pt[:])
            else:
                nc.vector.tensor_scalar_mul(out_ap, pt[:], scale)

        # ------- stage 1: x_wT = (x@W+b).T [D,N]; xthT = (x@th).T [1,N] ----
        xT_t = med.tile([D, N], bf16, tag="xT")
        nc.sync.dma_start(out=xT_t[:], in_=xT_d.ap())
        x_wT = med.tile([D, N], fp32, tag="x_wT")
        xthT = med.tile([1, N], fp32, tag="xthT")
        for blk in range(N // F):
            sl = slice(blk * F, (blk + 1) * F)
            p1 = ps.tile([128, F], fp32, tag="ps", name="p1")
            nc.tensor.matmul(p1[:], w_t[:], xT_t[:, sl], start=True, stop=True)
            nc.vector.tensor_scalar_add(x_wT[:, sl], p1[:], b_t[:])
            p2 = ps.tile([1, F], fp32, tag="ps", name="p2")
            nc.tensor.matmul(p2[:], th_t[:], xT_t[:, sl], start=True, stop=True)
            nc.vector.tensor_copy(xthT[:, sl], p2[:])

        # x_w vN tiles [128n, 128d] packed as x_wv[:, j*128:...]
        x_wv = med.tile([D, N], bf16, tag="x_wv")
        for j in range(NT):
            transpose_cols(x_wT[:], j, x_wv[:, j * 128:(j + 1) * 128])
        # xth vN columns, hi/lo bf16 split so scores keep ~fp32 accuracy
        xthv = med.tile([128, 2 * NT], bf16, tag="xthv")
        xthv32 = med.tile([128, 1], fp32, tag="xthv32")
        for j in range(NT):
            pt = ps.tile([128, 1], fp32, tag="ps", name="ptx")
            nc.tensor.matmul(pt[:], xthT[:, j * 128:(j + 1) * 128], one11[:],
                             start=True, stop=True)
            nc.vector.tensor_copy(xthv[:, 2 * j:2 * j + 1], pt[:])
            nc.vector.tensor_tensor(xthv32[:], pt[:], xthv[:, 2 * j:2 * j + 1],
                                    op=Alu.subtract)
            nc.vector.tensor_copy(xthv[:, 2 * j + 1:2 * j + 2], xthv32[:])

        # ------- generic streamed stage: acc[d, own] += stat(j).T @ M[j] ---
        def stream_stage(dram_t, stat_fn, name, extra=None):
            accs = [ps.tile([128, F], fp32, tag="ps", name=f"{name}_a{i}")
                    for i in range(HALF // F)]
            for c in range(NCHUNK):
                chk = stream.tile([128, CH * HALF], bf16, tag="stream",
                                  name="chk")
                nc.sync.dma_start(
                    out=chk[:].rearrange("p (g n) -> p g n", g=CH),
                    in_=dram_t.ap()[c * CH * 128:(c + 1) * CH * 128, :]
                        .rearrange("(g p) n -> p g n", g=CH))
                for jj in range(CH):
                    j = c * CH + jj
                    st, sp = (j == 0), (j == NT - 1)
                    for blk in range(HALF // F):
                        sl = slice(jj * HALF + blk * F,
                                   jj * HALF + (blk + 1) * F)
                        nc.tensor.matmul(accs[blk][:], stat_fn(j), chk[:, sl],
                                         start=st, stop=sp)
                    if extra is not None:
                        extra(j, chk, jj)
            return accs

        # ------- exchange: pair-AllGather of an own-half bf16 tile ---------
        def exchange(ex_ap, width, name, fulltag):
            ci = dram.tile([128, width], bf16, tag=f"{name}_i", name=f"{name}_i")
            co = dram.tile([2 * 128, width], bf16, tag=f"{name}_o",
                           name=f"{name}_o")
            nc.sync.dma_start(out=ci[:], in_=ex_ap)
            nc.gpsimd.collective_compute(
                "AllGather", Alu.bypass, replica_groups=PAIRS,
                ins=[ci.opt()], outs=[co.opt()])
            full = med.tile([128, 2 * width], bf16, tag=fulltag,
                            name=f"{name}_f")
            nc.sync.dma_start(
                out=full[:].rearrange("p (g n) -> p g n", g=2),
                in_=co[:].rearrange("(g p) n -> p g n", g=2))
            return full

        # ------- stage 2: hxT[d, e'] = (Ht@x_w).T own e-half; scores -------
        st_ps = [ps.tile([2, F], fp32, tag="ps", name=f"st{i}")
                 for i in range(HALF // F)]

        def s2_extra(j, chk, jj):
            st, sp = (j == 0), (j == NT - 1)
            for blk in range(HALF // F):
                sl = slice(jj * HALF + blk * F, jj * HALF + (blk + 1) * F)
                nc.tensor.matmul(st_ps[blk][:], xthv[:, 2 * j:2 * j + 2],
                                 chk[:, sl], start=st, stop=sp)

        hx_ps = stream_stage(
            hcol_d, lambda j: x_wv[:, j * 128:(j + 1) * 128], "hx",
            extra=s2_extra)
        hxT = med.tile([D, HALF], fp32, tag="hxT")
        sth = small.tile([1, HALF], fp32, tag="sth")
        for blk in range(HALF // F):
            sl = slice(blk * F, (blk + 1) * F)
            nc.vector.tensor_copy(hxT[:, sl], hx_ps[blk][:])
            s2sb = med.tile([2, F], fp32, tag="s2sb", name=f"s2sb{blk}")
            nc.vector.tensor_copy(s2sb[:], st_ps[blk][0:2, :])
            sp = ps.tile([1, F], fp32, tag="ps", name=f"sp{blk}")
            nc.tensor.matmul(sp[:], ones2[:], s2sb[:], start=True, stop=True)
            nc.vector.tensor_copy(sth[:, sl], sp[:])

        # softmax pieces: attn_u = exp(sth)*mask ; z_own = sum(attn_u)
        attn_u = small.tile([1, HALF], fp32, tag="attn_u")
        nc.scalar.activation(attn_u[:], sth[:], Act.Exp)
        nc.vector.tensor_mul(attn_u[:], attn_u[:], mask_t[:])
        z_t = small.tile([1, 1], fp32, tag="z_t")
        nc.vector.reduce_sum(z_t[:], attn_u[:], axis=mybir.AxisListType.X)
        # attn as per-partition columns attnv[:, t]
        attnv = med.tile([128, HT], fp32, tag="attnv")
        for t in range(HT):
            pt = ps.tile([128, 1], fp32, tag="ps", name="pta")
            nc.tensor.matmul(pt[:], attn_u[:, t * 128:(t + 1) * 128], one11[:],
                             start=True, stop=True)
            nc.vector.tensor_copy(attnv[:, t:t + 1], pt[:])
        # eps-scaled hxT for stage 10
        ehxT = med.tile([D, HALF], fp32, tag="ehxT")
        nc.vector.tensor_scalar_mul(ehxT[:], hxT[:], eps_t[:])

        # h1a own vE tiles (bf16), z hi/lo riding in cols HALF..HALF+1
        exa = med.tile([128, HALF + ZW], bf16, tag="exown")
        for t in range(HT):
            transpose_cols(hxT[:], t, exa[:, t * 128:(t + 1) * 128],
                           scale=attnv[:, t:t + 1])
        nc.vector.memset(exa[:, HALF:], 0.0)
        nc.vector.tensor_copy(exa[0:1, HALF:HALF + 1], z_t[:])
        zt2 = small.tile([1, 1], fp32, tag="zt2")
        nc.vector.tensor_copy(zt2[:], exa[0:1, HALF:HALF + 1])
        nc.vector.tensor_tensor(zt2[:], z_t[:], zt2[:], op=Alu.subtract)
        nc.vector.tensor_copy(exa[0:1, HALF + 1:HALF + 2], zt2[:])

        h1af = exchange(exa[:], HALF + ZW, "ex1", "fullB")

        # rz = 1 / (z_own + z_partner), broadcast to [128, 1]
        za = small.tile([1, 2], fp32, tag="za")
        zb = small.tile([1, 2], fp32, tag="zb")
        nc.vector.tensor_copy(za[:], h1af[0:1, HALF:HALF + 2])
        nc.vector.tensor_copy(zb[:], h1af[0:1, (HALF + ZW) + HALF:
                                          (HALF + ZW) + HALF + 2])
        nc.vector.tensor_tensor(za[:], za[:], zb[:], op=Alu.add)
        zs = small.tile([1, 1], fp32, tag="zs")
        nc.vector.reduce_sum(zs[:], za[:], axis=mybir.AxisListType.X)
        rz = small.tile([1, 1], fp32, tag="rz")
        nc.vector.reciprocal(rz[:], zs[:])
        rz_ps = ps.tile([128, 1], fp32, tag="ps", name="rzp")
        nc.tensor.matmul(rz_ps[:], ones_row[:], rz[:], start=True, stop=True)
        rz_bc = small.tile([128, 1], fp32, tag="rz_bc")
        nc.vector.tensor_copy(rz_bc[:], rz_ps[:])

        def h1a_tile(j):
            g, t = divmod(j, HT)
            off = g * (HALF + ZW) + t * 128
            return h1af[:, off:off + 128]

        # evict a stage's psum accs to fp32 SBUF, transpose to bf16 own tile
        def evict(accs, name, scale=None, add=None):
            oT = med.tile([D, HALF], fp32, tag="oT32", name=f"{name}_oT")
            for blk in range(HALF // F):
                sl = slice(blk * F, (blk + 1) * F)
                if add is None:
                    nc.vector.tensor_copy(oT[:, sl], accs[blk][:])
                else:
                    nc.vector.tensor_tensor(oT[:, sl], accs[blk][:],
                                            add[:, sl], op=Alu.add)
            ex = med.tile([128, HALF], bf16, tag="exown2", name=f"{name}_ex")
            for t in range(HT):
                transpose_cols(oT[:], t, ex[:, t * 128:(t + 1) * 128],
                               scale=scale)
            return ex

        # ------- stage 6: h1b own n-half = (H @ h1a)/z -------
        accs = stream_stage(htc_d, h1a_tile, "h1b")
        exb = evict(accs, "h1b", scale=rz_bc[:])
        h1bv = exchange(exb[:], HALF, "ex2", "fullA")

        # ------- stage 7: h1c own n-half = Dv @ h1b -------
        accs = stream_stage(dvT_d, lambda j: h1bv[:, j * 128:(j + 1) * 128],
                            "h1c")
        exc = evict(accs, "h1c")
        h1cv = exchange(exc[:], HALF, "ex3", "fullB")

        # ------- stage 8: h1d own e-half = Ht @ h1c -------
        accs = stream_stage(hcol_d, lambda j: h1cv[:, j * 128:(j + 1) * 128],
                            "h1d")
        exd = evict(accs, "h1d")
        h1dv = exchange(exd[:], HALF, "ex4", "fullA")

        # ------- stage 9+10: h own e-half = De @ h1d + eps*hx -------
        accs = stream_stage(deT_d, lambda j: h1dv[:, j * 128:(j + 1) * 128],
                            "h1e")
        exe = evict(accs, "h1e", add=ehxT)
        hv = exchange(exe[:], HALF, "ex5", "fullB")

        # ------- stage 11: out own n-half = H @ h -------
        accs = stream_stage(htc_d, lambda j: hv[:, j * 128:(j + 1) * 128],
                            "out")
        outT = med.tile([D, HALF], fp32, tag="oT32", name="outT")
        for blk in range(HALF // F):
            sl = slice(blk * F, (blk + 1) * F)
            nc.vector.tensor_copy(outT[:, sl], accs[blk][:])

        # ------- stage 12: epilogue: bn(leaky_relu(outT)) -----------------
        nc.scalar.activation(outT[:], outT[:], Act.Lrelu, alpha=0.01)
        nc.vector.tensor_scalar(outT[:], outT[:], s_bn[:], t_bn[:],
                                op0=Alu.mult, op1=Alu.add)
        nc.sync.dma_start(out=y_d.ap(), in_=outT[:])

    nc.finalize()
    return nc


def _get_nc():
    if "nc" not in _CACHE:
        _CACHE["nc"] = _build()
    return _CACHE["nc"]


def _shard(inputs):
    import ml_dtypes
    bf16 = ml_dtypes.bfloat16

    H = np.asarray(inputs["incident_mat"], dtype=np.float32)
    Dv = np.asarray(inputs["degree_v"], dtype=np.float32)
    De = np.asarray(inputs["degree_e"], dtype=np.float32)
    x = np.asarray(inputs["x"], dtype=np.float32)
    em = np.asarray(inputs["e_masks"])
    w = np.asarray(inputs["mlp_W"], dtype=np.float32).astype(bf16)
    b = np.ascontiguousarray(
        np.asarray(inputs["mlp_b"], dtype=np.float32).reshape(D, 1))
    th = np.asarray(inputs["theta_att"],
                    dtype=np.float32).reshape(D, 1).astype(bf16)
    eps = np.full((D, 1), float(np.asarray(inputs["eps"]).reshape(-1)[0]),
                  dtype=np.float32)

    def col(v):
        return np.ascontiguousarray(
            np.asarray(v, dtype=np.float32).reshape(D, 1))

    bng, bnb = col(inputs["bn_gamma"]), col(inputs["bn_beta"])
    bnm, bnv = col(inputs["bn_mean"]), col(inputs["bn_var"])

    in_maps = []
    for core in range(NCORES):
        g, c = core // 2, core % 2
        lo, hi = c * HALF, (c + 1) * HALF
        Hg = H[g]
        in_maps.append({
            "xT": np.ascontiguousarray(x[g].T).astype(bf16),
            "hcol": np.ascontiguousarray(Hg[:, lo:hi]).astype(bf16),
            "htc": np.ascontiguousarray(Hg[lo:hi, :].T).astype(bf16),
            "dvT": np.ascontiguousarray(Dv[g][lo:hi, :].T).astype(bf16),
            "deT": np.ascontiguousarray(De[g][lo:hi, :].T).astype(bf16),
            "w": w, "b": b, "th": th,
            "mask": np.ascontiguousarray(
                em[g, lo:hi].astype(np.float32).reshape(1, HALF)),
            "eps": eps,
            "bng": bng, "bnb": bnb, "bnm": bnm, "bnv": bnv,
        })
    return in_maps


def kernel(**inputs):
    from concourse.bass_utils import run_bass_kernel_spmd

    nc = _get_nc()
    in_maps = _shard(inputs)
    res = run_bass_kernel_spmd(nc, in_maps, list(range(NCORES)))
    out = np.empty((B, N, D), dtype=np.float32)
    for core in range(NCORES):
        g, c = core // 2, core % 2
        lo, hi = c * HALF, (c + 1) * HALF
        out[g, lo:hi, :] = res.results[core]["y"].T
    return out


# revision 8
# speedup vs baseline: 1.8253x; 1.0339x over previous
"""HGNN layer (hypergraph message passing) Trainium2 kernel, 8 NeuronCores.

Sharding: one graph per PAIR of cores (4 graphs x 2 cores), output-split:
within a pair each core owns HALF of every stage's output rows/columns
(e-half for Ht@/De@ stages, n-half for H@/Dv@ stages). Each stage streams
its big-matrix shard (bf16, cast on host) as the MOVING matmul operand in
2MB chunks over HWDGE while the [*,128] intermediate sits stationary in
bf16 vN/vE tile form. Between stages, the pair exchanges the half-outputs
with a 0.5MB AllGather (the softmax normalizer z rides along in the first
exchange as a hi/lo bf16 pair). Only 4 distinct big tensors are uploaded
per core (hcol, htc, dvT, deT); hcol and htc are each streamed twice.
"""

import numpy as np

B, N, E, D = 4, 4096, 4096, 128
HALF = N // 2
NCORES = 8
PAIRS = [[0, 1], [2, 3], [4, 5], [6, 7]]
BN_EPS = 1e-5
F = 512                 # moving free-dim per matmul
NT = N // 128           # 32 k-tiles over a full 4096 dim
HT = HALF // 128        # 16 tiles over a half (own output)
CH = 8                  # k-tiles per DMA chunk (4MB bf16)
NCHUNK = NT // CH       # 4 chunks per streamed matrix
ZW = 8                  # rider columns appended to exchange 1

_CACHE = {}


def _build():
    import concourse.bacc as bacc
    import concourse.mybir as mybir
    import concourse.tile as tile
    from concourse.masks import make_identity
    from contextlib import ExitStack

    fp32 = mybir.dt.float32
    bf16 = mybir.dt.bfloat16
    Act = mybir.ActivationFunctionType
    Alu = mybir.AluOpType

    nc = bacc.Bacc("TRN2", target_bir_lowering=False, debug=False,
                   num_devices=NCORES)

    # ---- per-core DRAM inputs (shards; see kernel() for host layout) ----
    # x_w = x@W+b and xth = x@theta are precomputed on host; x_wv is the
    # packed vN tile form [128, NT*128], xthv the hi/lo bf16 split pairs.
    xwv_d = nc.dram_tensor("xwv", [128, NT * 128], bf16, kind="ExternalInput")
    xthv_d = nc.dram_tensor("xthv", [128, 2 * NT], bf16, kind="ExternalInput")
    hcol_d = nc.dram_tensor("hcol", [N, HALF], bf16, kind="ExternalInput")
    htc_d = nc.dram_tensor("htc", [E, HALF], bf16, kind="ExternalInput")
    dvT_d = nc.dram_tensor("dvT", [N, HALF], bf16, kind="ExternalInput")
    deT_d = nc.dram_tensor("deT", [E, HALF], bf16, kind="ExternalInput")
    mask_d = nc.dram_tensor("mask", [1, HALF], fp32, kind="ExternalInput")
    eps_d = nc.dram_tensor("eps", [D, 1], fp32, kind="ExternalInput")
    bng_d = nc.dram_tensor("bng", [D, 1], fp32, kind="ExternalInput")
    bnb_d = nc.dram_tensor("bnb", [D, 1], fp32, kind="ExternalInput")
    bnm_d = nc.dram_tensor("bnm", [D, 1], fp32, kind="ExternalInput")
    bnv_d = nc.dram_tensor("bnv", [D, 1], fp32, kind="ExternalInput")
    y_d = nc.dram_tensor("y", [D, HALF], fp32, kind="ExternalOutput")

    with tile.TileContext(nc) as tc, ExitStack() as ctx:
        const = ctx.enter_context(tc.tile_pool(name="const", bufs=1))
        stream = ctx.enter_context(tc.tile_pool(name="stream", bufs=3))
        med = ctx.enter_context(tc.tile_pool(name="med", bufs=1))
        small = ctx.enter_context(tc.tile_pool(name="small", bufs=1))
        ps = ctx.enter_context(tc.tile_pool(name="ps", bufs=8, space="PSUM"))
        dram = ctx.enter_context(tc.tile_pool(name="dram", bufs=1, space="DRAM"))

        ident = const.tile([128, 128], fp32)
        make_identity(nc, ident)
        one11 = const.tile([1, 1], fp32)
        nc.vector.memset(one11[:], 1.0)
        ones_row = const.tile([1, 128], fp32)
        nc.vector.memset(ones_row[:], 1.0)
        ones2 = const.tile([2, 1], fp32)
        nc.vector.memset(ones2[:], 1.0)

        def load_param(dt_, dt=fp32):
            t = const.tile([D, 1], dt, tag=dt_.name + "_p")
            nc.sync.dma_start(out=t[:], in_=dt_.ap())
            return t

        eps_t = load_param(eps_d)
        bng_t = load_param(bng_d)
        bnb_t = load_param(bnb_d)
        bnm_t = load_param(bnm_d)
        bnv_t = load_param(bnv_d)
        mask_t = const.tile([1, HALF], fp32)
        nc.sync.dma_start(out=mask_t[:], in_=mask_d.ap())

        # bn scale s = gamma * rsqrt(var + eps_bn); shift t = beta - mean*s
        s_bn = small.tile([D, 1], fp32, tag="s_bn")
        nc.vector.tensor_scalar_add(s_bn[:], bnv_t[:], BN_EPS)
        nc.scalar.activation(s_bn[:], s_bn[:], Act.Sqrt)
        nc.vector.reciprocal(s_bn[:], s_bn[:])
        nc.vector.tensor_mul(s_bn[:], s_bn[:], bng_t[:])
        t_bn = small.tile([D, 1], fp32, tag="t_bn")
        nc.vector.tensor_mul(t_bn[:], bnm_t[:], s_bn[:])
        nc.vector.tensor_tensor(t_bn[:], bnb_t[:], t_bn[:], op=Alu.subtract)

        def transpose_cols(src, j, out_ap, scale=None):
            """PE-transpose src[:, 128j:128(j+1)] -> out_ap (optionally
            scaled per-partition by `scale` [128,1]) via psum."""
            pt = ps.tile([128, 128], fp32, tag="ps", name="pt")
            nc.tensor.transpose(pt[:], src[:, j * 128:(j + 1) * 128], ident[:])
            if scale is None:
                nc.vector.tensor_copy(out_ap, pt[:])
            else:
                nc.vector.tensor_scalar_mul(out_ap, pt[:], scale)

        # ------- stage 1 (host-precomputed): load x_wv vN tiles + xth hi/lo
        x_wv = med.tile([D, N], bf16, tag="x_wv")
        nc.sync.dma_start(out=x_wv[:], in_=xwv_d.ap())
        xthv = med.tile([128, 2 * NT], bf16, tag="xthv")
        nc.sync.dma_start(out=xthv[:], in_=xthv_d.ap())

        # ------- generic streamed stage: acc[d, own] += stat(j).T @ M[j] ---
        def stream_stage(dram_t, stat_fn, name, extra=None):
            accs = [ps.tile([128, F], fp32, tag="ps", name=f"{name}_a{i}")
                    for i in range(HALF // F)]
            for c in range(NCHUNK):
                chk = stream.tile([128, CH * HALF], bf16, tag="stream",
                                  name="chk")
                nc.sync.dma_start(
                    out=chk[:].rearrange("p (g n) -> p g n", g=CH),
                    in_=dram_t.ap()[c * CH * 128:(c + 1) * CH * 128, :]
                        .rearrange("(g p) n -> p g n", g=CH))
                for jj in range(CH):
                    j = c * CH + jj
                    st, sp = (j == 0), (j == NT - 1)
                    for blk in range(HALF // F):
                        sl = slice(jj * HALF + blk * F,
                                   jj * HALF + (blk + 1) * F)
                        nc.tensor.matmul(accs[blk][:], stat_fn(j), chk[:, sl],
                                         start=st, stop=sp)
                    if extra is not None:
                        extra(j, chk, jj)
            return accs

        # ------- exchange: pair-AllGather of an own-half bf16 tile ---------
        def exchange(ex_ap, width, name, fulltag):
            ci = dram.tile([128, width], bf16, tag=f"{name}_i", name=f"{name}_i")
            co = dram.tile([2 * 128, width], bf16, tag=f"{name}_o",
                           name=f"{name}_o")
            nc.sync.dma_start(out=ci[:], in_=ex_ap)
            nc.gpsimd.collective_compute(
                "AllGather", Alu.bypass, replica_groups=PAIRS,
                ins=[ci.opt()], outs=[co.opt()])
            full = med.tile([128, 2 * width], bf16, tag=fulltag,
                            name=f"{name}_f")
            nc.sync.dma_start(
                out=full[:].rearrange("p (g n) -> p g n", g=2),
                in_=co[:].rearrange("(g p) n -> p g n", g=2))
            return full

        # ------- stage 2: hxT[d, e'] = (Ht@x_w).T own e-half; scores -------
        st_ps = [ps.tile([2, F], fp32, tag="ps", name=f"st{i}")
                 for i in range(HALF // F)]

        def s2_extra(j, chk, jj):
            st, sp = (j == 0), (j == NT - 1)
            for blk in range(HALF // F):
                sl = slice(jj * HALF + blk * F, jj * HALF + (blk + 1) * F)
                nc.tensor.matmul(st_ps[blk][:], xthv[:, 2 * j:2 * j + 2],
                                 chk[:, sl], start=st, stop=sp)

        hx_ps = stream_stage(
            hcol_d, lambda j: x_wv[:, j * 128:(j + 1) * 128], "hx",
            extra=s2_extra)
        hxT = med.tile([D, HALF], fp32, tag="hxT")
        sth = small.tile([1, HALF], fp32, tag="sth")
        for blk in range(HALF // F):
            sl = slice(blk * F, (blk + 1) * F)
            nc.vector.tensor_copy(hxT[:, sl], hx_ps[blk][:])
            s2sb = med.tile([2, F], fp32, tag="s2sb", name=f"s2sb{blk}")
            nc.vector.tensor_copy(s2sb[:], st_ps[blk][0:2, :])
            sp = ps.tile([1, F], fp32, tag="ps", name=f"sp{blk}")
            nc.tensor.matmul(sp[:], ones2[:], s2sb[:], start=True, stop=True)
            nc.vector.tensor_copy(sth[:, sl], sp[:])

        # softmax pieces: attn_u = exp(sth)*mask ; z_own = sum(attn_u)
        attn_u = small.tile([1, HALF], fp32, tag="attn_u")
        nc.scalar.activation(attn_u[:], sth[:], Act.Exp)
        nc.vector.tensor_mul(attn_u[:], attn_u[:], mask_t[:])
        z_t = small.tile([1, 1], fp32, tag="z_t")
        nc.vector.reduce_sum(z_t[:], attn_u[:], axis=mybir.AxisListType.X)
        # attn as per-partition columns attnv[:, t]
        attnv = med.tile([128, HT], fp32, tag="attnv")
        for t in range(HT):
            pt = ps.tile([128, 1], fp32, tag="ps", name="pta")
            nc.tensor.matmul(pt[:], attn_u[:, t * 128:(t + 1) * 128], one11[:],
                             start=True, stop=True)
            nc.vector.tensor_copy(attnv[:, t:t + 1], pt[:])
        # eps-scaled hxT for stage 10
        ehxT = med.tile([D, HALF], fp32, tag="ehxT")
        nc.vector.tensor_scalar_mul(ehxT[:], hxT[:], eps_t[:])

        # h1a own vE tiles (bf16), z hi/lo riding in cols HALF..HALF+1
        exa = med.tile([128, HALF + ZW], bf16, tag="exown")
        for t in range(HT):
            transpose_cols(hxT[:], t, exa[:, t * 128:(t + 1) * 128],
                           scale=attnv[:, t:t + 1])
        nc.vector.memset(exa[:, HALF:], 0.0)
        nc.vector.tensor_copy(exa[0:1, HALF:HALF + 1], z_t[:])
        zt2 = small.tile([1, 1], fp32, tag="zt2")
        nc.vector.tensor_copy(zt2[:], exa[0:1, HALF:HALF + 1])
        nc.vector.tensor_tensor(zt2[:], z_t[:], zt2[:], op=Alu.subtract)
        nc.vector.tensor_copy(exa[0:1, HALF + 1:HALF + 2], zt2[:])

        h1af = exchange(exa[:], HALF + ZW, "ex1", "fullB")

        # rz = 1 / (z_own + z_partner), broadcast to [128, 1]
        za = small.tile([1, 2], fp32, tag="za")
        zb = small.tile([1, 2], fp32, tag="zb")
        nc.vector.tensor_copy(za[:], h1af[0:1, HALF:HALF + 2])
        nc.vector.tensor_copy(zb[:], h1af[0:1, (HALF + ZW) + HALF:
                                          (HALF + ZW) + HALF + 2])
        nc.vector.tensor_tensor(za[:], za[:], zb[:], op=Alu.add)
        zs = small.tile([1, 1], fp32, tag="zs")
        nc.vector.reduce_sum(zs[:], za[:], axis=mybir.AxisListType.X)
        rz = small.tile([1, 1], fp32, tag="rz")
        nc.vector.reciprocal(rz[:], zs[:])
        rz_ps = ps.tile([128, 1], fp32, tag="ps", name="rzp")
        nc.tensor.matmul(rz_ps[:], ones_row[:], rz[:], start=True, stop=True)
        rz_bc = small.tile([128, 1], fp32, tag="rz_bc")
        nc.vector.tensor_copy(rz_bc[:], rz_ps[:])

        def h1a_tile(j):
            g, t = divmod(j, HT)
            off = g * (HALF + ZW) + t * 128
            return h1af[:, off:off + 128]

        # evict a stage's psum accs to fp32 SBUF, transpose to bf16 own tile
        def evict(accs, name, scale=None, add=None):
            oT = med.tile([D, HALF], fp32, tag="oT32", name=f"{name}_oT")
            for blk in range(HALF // F):
                sl = slice(blk * F, (blk + 1) * F)
                if add is None:
                    nc.vector.tensor_copy(oT[:, sl], accs[blk][:])
                else:
                    nc.vector.tensor_tensor(oT[:, sl], accs[blk][:],
                                            add[:, sl], op=Alu.add)
            ex = med.tile([128, HALF], bf16, tag="exown2", name=f"{name}_ex")
            for t in range(HT):
                transpose_cols(oT[:], t, ex[:, t * 128:(t + 1) * 128],
                               scale=scale)
            return ex

        # ------- stage 6: h1b own n-half = (H @ h1a)/z -------
        accs = stream_stage(htc_d, h1a_tile, "h1b")
        exb = evict(accs, "h1b", scale=rz_bc[:])
        h1bv = exchange(exb[:], HALF, "ex2", "fullA")

        # ------- stage 7: h1c own n-half = Dv @ h1b -------
        accs = stream_stage(dvT_d, lambda j: h1bv[:, j * 128:(j + 1) * 128],
                            "h1c")
        exc = evict(accs, "h1c")
        h1cv = exchange(exc[:], HALF, "ex3", "fullB")

        # ------- stage 8: h1d own e-half = Ht @ h1c -------
        accs = stream_stage(hcol_d, lambda j: h1cv[:, j * 128:(j + 1) * 128],
                            "h1d")
        exd = evict(accs, "h1d")
        h1dv = exchange(exd[:], HALF, "ex4", "fullA")

        # ------- stage 9+10: h own e-half = De @ h1d + eps*hx -------
        accs = stream_stage(deT_d, lambda j: h1dv[:, j * 128:(j + 1) * 128],
                            "h1e")
        exe = evict(accs, "h1e", add=ehxT)
        hv = exchange(exe[:], HALF, "ex5", "fullB")

        # ------- stage 11: out own n-half = H @ h -------
        accs = stream_stage(htc_d, lambda j: hv[:, j * 128:(j + 1) * 128],
                            "out")
        # ------- stage 12: epilogue: bn(leaky_relu(out)), blockwise -------
        outT = med.tile([D, HALF], fp32, tag="oT32", name="outT")
        for blk in range(HALF // F):
            sl = slice(blk * F, (blk + 1) * F)
            nc.scalar.activation(outT[:, sl], accs[blk][:], Act.Lrelu,
                                 alpha=0.01)
            nc.vector.tensor_scalar(outT[:, sl], outT[:, sl], s_bn[:],
                                    t_bn[:], op0=Alu.mult, op1=Alu.add)
            nc.sync.dma_start(out=y_d.ap()[:, sl], in_=outT[:, sl])

    nc.finalize()
    return nc


def _get_nc():
    if "nc" not in _CACHE:
        _CACHE["nc"] = _build()
    return _CACHE["nc"]


def _shard(inputs):
    import ml_dtypes
    bf16 = ml_dtypes.bfloat16

    H = np.asarray(inputs["incident_mat"], dtype=np.float32)
    Dv = np.asarray(inputs["degree_v"], dtype=np.float32)
    De = np.asarray(inputs["degree_e"], dtype=np.float32)
    x = np.asarray(inputs["x"], dtype=np.float32)
    em = np.asarray(inputs["e_masks"])
    w = np.asarray(inputs["mlp_W"], dtype=np.float32)
    b = np.asarray(inputs["mlp_b"], dtype=np.float32)
    th = np.asarray(inputs["theta_att"], dtype=np.float32).reshape(D)
    eps = np.full((D, 1), float(np.asarray(inputs["eps"]).reshape(-1)[0]),
                  dtype=np.float32)

    def col(v):
        return np.ascontiguousarray(
            np.asarray(v, dtype=np.float32).reshape(D, 1))

    bng, bnb = col(inputs["bn_gamma"]), col(inputs["bn_beta"])
    bnm, bnv = col(inputs["bn_mean"]), col(inputs["bn_var"])

    in_maps = []
    for core in range(NCORES):
        g, c = core // 2, core % 2
        lo, hi = c * HALF, (c + 1) * HALF
        Hg = H[g]
        # host stage 1: x_w = x@W+b packed into vN tile form; xth hi/lo
        xw = x[g] @ w + b
        xwv = np.ascontiguousarray(
            xw.reshape(NT, 128, D).transpose(1, 0, 2).reshape(128, NT * D)
        ).astype(bf16)
        xth = (x[g] @ th).astype(np.float32)
        hi32 = xth.astype(bf16).astype(np.float32)
        xthv = np.empty((128, 2 * NT), dtype=bf16)
        xthv[:, 0::2] = xth.astype(bf16).reshape(NT, 128).T
        xthv[:, 1::2] = (xth - hi32).astype(bf16).reshape(NT, 128).T
        in_maps.append({
            "xwv": xwv,
            "xthv": xthv,
            "hcol": np.ascontiguousarray(Hg[:, lo:hi]).astype(bf16),
            "htc": np.ascontiguousarray(Hg[lo:hi, :].T).astype(bf16),
            "dvT": np.ascontiguousarray(Dv[g][lo:hi, :].T).astype(bf16),
            "deT": np.ascontiguousarray(De[g][lo:hi, :].T).astype(bf16),
            "mask": np.ascontiguousarray(
                em[g, lo:hi].astype(np.float32).reshape(1, HALF)),
            "eps": eps,
            "bng": bng, "bnb": bnb, "bnm": bnm, "bnv": bnv,
        })
    return in_maps


def kernel(**inputs):
    from concourse.bass_utils import run_bass_kernel_spmd

    nc = _get_nc()
    in_maps = _shard(inputs)
    res = run_bass_kernel_spmd(nc, in_maps, list(range(NCORES)))
    out = np.empty((B, N, D), dtype=np.float32)
    for core in range(NCORES):
        g, c = core // 2, core % 2
        lo, hi = c * HALF, (c + 1) * HALF
        out[g, lo:hi, :] = res.results[core]["y"].T
    return out


# revision 15
# speedup vs baseline: 2.1121x; 1.1571x over previous
"""HGNN layer (hypergraph message passing) Trainium2 kernel, 8 NeuronCores.

Sharding: one graph per PAIR of cores (4 graphs x 2 cores), output-split:
within a pair each core owns HALF of every stage's output rows/columns.
Each stage streams its big-matrix shard as the MOVING matmul operand in
2MB chunks over HWDGE while the [*,128] intermediate sits stationary in
bf16 vN/vE tile form. The incidence matrix H is 0/1 so its two layouts
(hcol, htc) stream as fp8_e4m3 (exact, half the bytes); degree matrices
stream bf16. All streamed tensors are row-permuted OWN-HALF-FIRST on the
host so each stage accumulates its own 16 k-tiles while the pair exchange
for the partner half is still in flight. Exchanges are pair AllReduces of
the own-half bf16 tile; the partner half is recovered symmetrically as
(sum - own). The softmax normalizer z rides in the first exchange as a
hi/lo bf16 pair. x@W+b and x@theta are precomputed on the host.
"""

import numpy as np

B, N, E, D = 4, 4096, 4096, 128
HALF = N // 2
NCORES = 8
PAIRS = [[0, 1], [2, 3], [4, 5], [6, 7]]
BN_EPS = 1e-5
F = 512                 # moving free-dim per matmul
NT = N // 128           # 32 k-tiles over a full 4096 dim
HT = HALF // 128        # 16 tiles over a half (own output)
CH8 = 8                 # k-tiles per fp8 chunk (2MB)
CH16 = 4                # k-tiles per bf16 chunk (2MB)
ZW = 8                  # rider columns appended to exchange 1

_CACHE = {}


def _build():
    import concourse.bacc as bacc
    import concourse.mybir as mybir
    import concourse.tile as tile
    from concourse.masks import make_identity
    from contextlib import ExitStack

    fp32 = mybir.dt.float32
    bf16 = mybir.dt.bfloat16
    fp8 = mybir.dt.float8e4
    Act = mybir.ActivationFunctionType
    Alu = mybir.AluOpType

    nc = bacc.Bacc("TRN2", target_bir_lowering=False, debug=False,
                   num_devices=NCORES)

    # ---- per-core DRAM inputs (shards; see kernel() for host layout) ----
    xwv_d = nc.dram_tensor("xwv", [128, NT * 128], bf16, kind="ExternalInput")
    xthv_d = nc.dram_tensor("xthv", [128, 2 * NT], bf16, kind="ExternalInput")
    hcol_d = nc.dram_tensor("hcol", [N, HALF], fp8, kind="ExternalInput")
    htc_d = nc.dram_tensor("htc", [E, HALF], fp8, kind="ExternalInput")
    dvT_d = nc.dram_tensor("dvT", [N, HALF], bf16, kind="ExternalInput")
    deT_d = nc.dram_tensor("deT", [E, HALF], bf16, kind="ExternalInput")
    mask_d = nc.dram_tensor("mask", [128, HT], fp32, kind="ExternalInput")
    eps_d = nc.dram_tensor("eps", [D, 1], fp32, kind="ExternalInput")
    bng_d = nc.dram_tensor("bng", [D, 1], fp32, kind="ExternalInput")
    bnb_d = nc.dram_tensor("bnb", [D, 1], fp32, kind="ExternalInput")
    bnm_d = nc.dram_tensor("bnm", [D, 1], fp32, kind="ExternalInput")
    bnv_d = nc.dram_tensor("bnv", [D, 1], fp32, kind="ExternalInput")
    y_d = nc.dram_tensor("y", [D, HALF], fp32, kind="ExternalOutput")

    with tile.TileContext(nc) as tc, ExitStack() as ctx:
        const = ctx.enter_context(tc.tile_pool(name="const", bufs=1))
        stream = ctx.enter_context(tc.tile_pool(name="stream", bufs=5))
        med = ctx.enter_context(tc.tile_pool(name="med", bufs=1))
        small = ctx.enter_context(tc.tile_pool(name="small", bufs=1))
        ps = ctx.enter_context(tc.tile_pool(name="ps", bufs=8, space="PSUM"))
        dram = ctx.enter_context(tc.tile_pool(name="dram", bufs=1, space="DRAM"))

        ident = const.tile([128, 128], fp32)
        make_identity(nc, ident)
        ones2 = const.tile([2, 1], fp32)
        nc.vector.memset(ones2[:], 1.0)
        onesc = const.tile([128, 1], fp32)
        nc.vector.memset(onesc[:], 1.0)
        ones_row = const.tile([1, 128], fp32)
        nc.vector.memset(ones_row[:], 1.0)

        def load_param(dt_):
            t = const.tile([D, 1], fp32, tag=dt_.name + "_p")
            nc.sync.dma_start(out=t[:], in_=dt_.ap())
            return t

        eps_t = load_param(eps_d)
        bng_t = load_param(bng_d)
        bnb_t = load_param(bnb_d)
        bnm_t = load_param(bnm_d)
        bnv_t = load_param(bnv_d)
        maskc = const.tile([128, HT], fp32)
        nc.sync.dma_start(out=maskc[:], in_=mask_d.ap())

        # bn scale s = gamma * rsqrt(var + eps_bn); shift t = beta - mean*s
        s_bn = small.tile([D, 1], fp32, tag="s_bn")
        nc.vector.tensor_scalar_add(s_bn[:], bnv_t[:], BN_EPS)
        nc.scalar.activation(s_bn[:], s_bn[:], Act.Sqrt)
        nc.vector.reciprocal(s_bn[:], s_bn[:])
        nc.vector.tensor_mul(s_bn[:], s_bn[:], bng_t[:])
        t_bn = small.tile([D, 1], fp32, tag="t_bn")
        nc.vector.tensor_mul(t_bn[:], bnm_t[:], s_bn[:])
        nc.vector.tensor_tensor(t_bn[:], bnb_t[:], t_bn[:], op=Alu.subtract)

        def transpose_cols(src, j, out_ap, scale=None):
            """PE-transpose src[:, 128j:128(j+1)] -> out_ap (optionally
            scaled per-partition by `scale` [128,1]) via psum."""
            pt = ps.tile([128, 128], fp32, tag="ps", name="pt")
            nc.tensor.transpose(pt[:], src[:, j * 128:(j + 1) * 128], ident[:])
            if scale is None:
                nc.vector.tensor_copy(out_ap, pt[:])
            else:
                nc.vector.tensor_scalar_mul(out_ap, pt[:], scale)

        # ------- stage 1 (host-precomputed): load x_wv vN tiles + xth hi/lo
        x_wv = med.tile([D, N], bf16, tag="x_wv")
        nc.sync.dma_start(out=x_wv[:], in_=xwv_d.ap())
        xthv = med.tile([128, 2 * NT], bf16, tag="xthv")
        nc.sync.dma_start(out=xthv[:], in_=xthv_d.ap())

        # ------- generic streamed stage: acc[d, own] += stat(j).T @ M[j] ---
        def stream_stage(dram_t, dt, ch, stat_fn, name, extra=None):
            accs = [ps.tile([128, F], fp32, tag="ps", name=f"{name}_a{i}")
                    for i in range(HALF // F)]
            for c in range(NT // ch):
                chk = stream.tile([128, ch * HALF], dt, tag="stream",
                                  name="chk")
                nc.sync.dma_start(
                    out=chk[:].rearrange("p (g n) -> p g n", g=ch),
                    in_=dram_t.ap()[c * ch * 128:(c + 1) * ch * 128, :]
                        .rearrange("(g p) n -> p g n", g=ch))
                for jj in range(ch):
                    j = c * ch + jj
                    st, sp = (j == 0), (j == NT - 1)
                    for blk in range(HALF // F):
                        sl = slice(jj * HALF + blk * F,
                                   jj * HALF + (blk + 1) * F)
                        nc.tensor.matmul(accs[blk][:], stat_fn(j), chk[:, sl],
                                         start=st, stop=sp)
                    if extra is not None:
                        extra(j, chk, jj)
            return accs

        # ------- exchange: pair-AllReduce of own-half bf16 tile ------------
        # Returns (own_ap_fn, partner tile); partner = sum - own.
        def exchange(ex, width, name, ab):
            ci = dram.tile([128, width], bf16, tag=f"{name}_i",
                           name=f"{name}_i")
            co = dram.tile([128, width], bf16, tag=f"{name}_o",
                           name=f"{name}_o")
            nc.sync.dma_start(out=ci[:], in_=ex[:])
            nc.gpsimd.collective_compute(
                "AllReduce", Alu.add, replica_groups=PAIRS,
                ins=[ci.opt()], outs=[co.opt()])
            summ = med.tile([128, width], bf16, tag=f"sum{ab}",
                            name=f"{name}_s")
            nc.sync.dma_start(out=summ[:], in_=co[:])
            part = med.tile([128, width], bf16, tag=f"pr{ab}",
                            name=f"{name}_pr")
            nc.vector.tensor_tensor(part[:], summ[:], ex[:], op=Alu.subtract)
            return part

        def split_stat(ex, part):
            def fn(j):
                src = ex if j < HT else part
                t = j % HT
                return src[:, t * 128:(t + 1) * 128]
            return fn

        # ------- stage 2: hxT[d, e'] = (Ht@x_w).T own e-half; scores -------
        st_ps = [ps.tile([2, F], fp32, tag="ps", name=f"st{i}")
                 for i in range(HALF // F)]

        def s2_extra(j, chk, jj):
            st, sp = (j == 0), (j == NT - 1)
            for blk in range(HALF // F):
                sl = slice(jj * HALF + blk * F, jj * HALF + (blk + 1) * F)
                nc.tensor.matmul(st_ps[blk][:], xthv[:, 2 * j:2 * j + 2],
                                 chk[:, sl], start=st, stop=sp)

        hx_ps = stream_stage(
            hcol_d, fp8, CH8,
            lambda j: x_wv[:, j * 128:(j + 1) * 128], "hx", extra=s2_extra)
        hxT = med.tile([D, HALF], fp32, tag="hxT")
        sth = small.tile([1, HALF], fp32, tag="sth")
        for blk in range(HALF // F):
            sl = slice(blk * F, (blk + 1) * F)
            nc.vector.tensor_copy(hxT[:, sl], hx_ps[blk][:])
            s2sb = med.tile([2, F], fp32, tag="s2sb", name=f"s2sb{blk}")
            nc.vector.tensor_copy(s2sb[:], st_ps[blk][0:2, :])
            sp = ps.tile([1, F], fp32, tag="ps", name=f"sp{blk}")
            nc.tensor.matmul(sp[:], ones2[:], s2sb[:], start=True, stop=True)
            nc.vector.tensor_copy(sth[:, sl], sp[:])

        # softmax in column form: attnv[p, t] = exp(score)*mask, z = sum
        # (bounce the score row through DRAM: SBUF partition dim is physical)
        sth_dr = dram.tile([1, HALF], fp32, tag="sth_dr", name="sth_dr")
        nc.sync.dma_start(out=sth_dr[:], in_=sth[:])
        s16 = med.tile([HT, 128], fp32, tag="s16")
        nc.sync.dma_start(out=s16[:],
                          in_=sth_dr[:].rearrange("o (t n) -> (o t) n", t=HT))
        spt = ps.tile([128, HT], fp32, tag="ps", name="spt")
        nc.tensor.transpose(spt[:, 0:HT], s16[:], ident[0:HT, 0:HT])
        attnv = med.tile([128, HT], fp32, tag="attnv")
        nc.scalar.activation(attnv[:], spt[:, 0:HT], Act.Exp)
        nc.vector.tensor_mul(attnv[:], attnv[:], maskc[:])
        zps = ps.tile([1, HT], fp32, tag="ps", name="zps")
        nc.tensor.matmul(zps[:], onesc[:], attnv[:], start=True, stop=True)
        zrow = small.tile([1, HT], fp32, tag="zrow")
        nc.vector.tensor_copy(zrow[:], zps[:])
        z_t = small.tile([1, 1], fp32, tag="z_t")
        nc.vector.reduce_sum(z_t[:], zrow[:], axis=mybir.AxisListType.X)

        # eps-scaled hxT for stage 10
        ehxT = med.tile([D, HALF], fp32, tag="ehxT")
        nc.vector.tensor_scalar_mul(ehxT[:], hxT[:], eps_t[:])

        # h1a own vE tiles (bf16), z hi/lo riding in cols HALF..HALF+1
        exa = med.tile([128, HALF + ZW], bf16, tag="exA")
        for t in range(HT):
            transpose_cols(hxT[:], t, exa[:, t * 128:(t + 1) * 128],
                           scale=attnv[:, t:t + 1])
        nc.vector.memset(exa[:, HALF:], 0.0)
        nc.vector.tensor_copy(exa[0:1, HALF:HALF + 1], z_t[:])
        zt2 = small.tile([1, 1], fp32, tag="zt2")
        nc.vector.tensor_copy(zt2[:], exa[0:1, HALF:HALF + 1])
        nc.vector.tensor_tensor(zt2[:], z_t[:], zt2[:], op=Alu.subtract)
        nc.vector.tensor_copy(exa[0:1, HALF + 1:HALF + 2], zt2[:])

        h1ap = exchange(exa, HALF + ZW, "ex1", "A")

        # rz = 1 / z_global (z summed by the AllReduce), broadcast [128, 1]
        za = small.tile([1, 2], fp32, tag="za")
        nc.vector.tensor_tensor(za[:], h1ap[0:1, HALF:HALF + 2],
                                exa[0:1, HALF:HALF + 2], op=Alu.add)
        zs = small.tile([1, 1], fp32, tag="zs")
        nc.vector.reduce_sum(zs[:], za[:], axis=mybir.AxisListType.X)
        rz = small.tile([1, 1], fp32, tag="rz")
        nc.vector.reciprocal(rz[:], zs[:])
        rz_ps = ps.tile([128, 1], fp32, tag="ps", name="rzp")
        nc.tensor.matmul(rz_ps[:], ones_row[:], rz[:], start=True, stop=True)
        rz_bc = small.tile([128, 1], fp32, tag="rz_bc")
        nc.vector.tensor_copy(rz_bc[:], rz_ps[:])

        # evict a stage's psum accs to fp32 SBUF, transpose to bf16 own tile
        def evict(accs, name, tag, scale=None, add=None, width=HALF):
            oT = med.tile([D, HALF], fp32, tag="oT32", name=f"{name}_oT")
            for blk in range(HALF // F):
                sl = slice(blk * F, (blk + 1) * F)
                if add is None:
                    nc.vector.tensor_copy(oT[:, sl], accs[blk][:])
                else:
                    nc.vector.tensor_tensor(oT[:, sl], accs[blk][:],
                                            add[:, sl], op=Alu.add)
            ex = med.tile([128, width], bf16, tag=tag, name=f"{name}_ex")
            for t in range(HT):
                transpose_cols(oT[:], t, ex[:, t * 128:(t + 1) * 128],
                               scale=scale)
            return ex

        # ------- stage 6: h1b own n-half = (H @ h1a)/z -------
        accs = stream_stage(htc_d, fp8, CH8, split_stat(exa, h1ap), "h1b")
        exb = evict(accs, "h1b", "exB", scale=rz_bc[:])
        h1bp = exchange(exb, HALF, "ex2", "B")

        # ------- stage 7: h1c own n-half = Dv @ h1b -------
        accs = stream_stage(dvT_d, bf16, CH16, split_stat(exb, h1bp), "h1c")
        exc = evict(accs, "h1c", "exA")
        h1cp = exchange(exc, HALF, "ex3", "A")

        # ------- stage 8: h1d own e-half = Ht @ h1c -------
        accs = stream_stage(hcol_d, fp8, CH8, split_stat(exc, h1cp), "h1d")
        exd = evict(accs, "h1d", "exB")
        h1dp = exchange(exd, HALF, "ex4", "B")

        # ------- stage 9+10: h own e-half = De @ h1d + eps*hx -------
        accs = stream_stage(deT_d, bf16, CH16, split_stat(exd, h1dp), "h1e")
        exe = evict(accs, "h1e", "exA", add=ehxT)
        hp = exchange(exe, HALF, "ex5", "A")

        # ------- stage 11: out own n-half = H @ h -------
        accs = stream_stage(htc_d, fp8, CH8, split_stat(exe, hp), "out")

        # ------- stage 12: epilogue: bn(leaky_relu(out)), blockwise -------
        outT = med.tile([D, HALF], fp32, tag="oT32", name="outT")
        for blk in range(HALF // F):
            sl = slice(blk * F, (blk + 1) * F)
            nc.scalar.activation(outT[:, sl], accs[blk][:], Act.Lrelu,
                                 alpha=0.01)
            nc.vector.tensor_scalar(outT[:, sl], outT[:, sl], s_bn[:],
                                    t_bn[:], op0=Alu.mult, op1=Alu.add)
            nc.sync.dma_start(out=y_d.ap()[:, sl], in_=outT[:, sl])

    nc.finalize()
    return nc


def _get_nc():
    if "nc" not in _CACHE:
        _CACHE["nc"] = _build()
    return _CACHE["nc"]


def _shard(inputs):
    import ml_dtypes
    bf16 = ml_dtypes.bfloat16
    fp8 = ml_dtypes.float8_e4m3

    H = np.asarray(inputs["incident_mat"], dtype=np.float32)
    Dv = np.asarray(inputs["degree_v"], dtype=np.float32)
    De = np.asarray(inputs["degree_e"], dtype=np.float32)
    x = np.asarray(inputs["x"], dtype=np.float32)
    em = np.asarray(inputs["e_masks"])
    w = np.asarray(inputs["mlp_W"], dtype=np.float32)
    b = np.asarray(inputs["mlp_b"], dtype=np.float32)
    th = np.asarray(inputs["theta_att"], dtype=np.float32).reshape(D)
    eps = np.full((D, 1), float(np.asarray(inputs["eps"]).reshape(-1)[0]),
                  dtype=np.float32)

    def col(v):
        return np.ascontiguousarray(
            np.asarray(v, dtype=np.float32).reshape(D, 1))

    bng, bnb = col(inputs["bn_gamma"]), col(inputs["bn_beta"])
    bnm, bnv = col(inputs["bn_mean"]), col(inputs["bn_var"])

    in_maps = []
    for core in range(NCORES):
        g, c = core // 2, core % 2
        lo, hi = c * HALF, (c + 1) * HALF
        # own-half-first row permutation for all streamed (contraction) dims
        perm = (np.r_[lo:hi, 0:lo] if c else np.r_[0:N])
        Hg = H[g]
        # host stage 1: x_w = x@W+b packed into vN tile form (perm rows)
        xw = (x[g] @ w + b)[perm]
        xwv = np.ascontiguousarray(
            xw.reshape(NT, 128, D).transpose(1, 0, 2).reshape(128, NT * D)
        ).astype(bf16)
        xth = (x[g] @ th).astype(np.float32)[perm]
        hi32 = xth.astype(bf16).astype(np.float32)
        xthv = np.empty((128, 2 * NT), dtype=bf16)
        xthv[:, 0::2] = xth.astype(bf16).reshape(NT, 128).T
        xthv[:, 1::2] = (xth - hi32).astype(bf16).reshape(NT, 128).T
        # mask in column-tile form [128, HT]
        maskc = np.ascontiguousarray(
            em[g, lo:hi].astype(np.float32).reshape(HT, 128).T)
        in_maps.append({
            "xwv": xwv,
            "xthv": xthv,
            "hcol": np.ascontiguousarray(Hg[perm][:, lo:hi]).astype(fp8),
            "htc": np.ascontiguousarray(Hg[lo:hi, :].T[perm]).astype(fp8),
            "dvT": np.ascontiguousarray(Dv[g][lo:hi, :].T[perm]).astype(bf16),
            "deT": np.ascontiguousarray(De[g][lo:hi, :].T[perm]).astype(bf16),
            "mask": maskc,
            "eps": eps,
            "bng": bng, "bnb": bnb, "bnm": bnm, "bnv": bnv,
        })
    return in_maps


def kernel(**inputs):
    from concourse.bass_utils import run_bass_kernel_spmd

    nc = _get_nc()
    in_maps = _shard(inputs)
    res = run_bass_kernel_spmd(nc, in_maps, list(range(NCORES)))
    out = np.empty((B, N, D), dtype=np.float32)
    for core in range(NCORES):
        g, c = core // 2, core % 2
        lo, hi = c * HALF, (c + 1) * HALF
        out[g, lo:hi, :] = res.results[core]["y"].T
    return out
